# revision 1
# baseline (speedup 1.0000x reference)
import numpy as np

B, N, DIM = 4, 4096, 1024
HEADS, DIM_HEAD, M = 16, 64, 128
DIM_INNER = HEADS * DIM_HEAD
SCALE = DIM_HEAD ** -0.5
HALVES = 2
NS = N // HALVES  # 2048 rows per shard


def _build():
    import jax
    import jax.numpy as jnp
    from jax.sharding import Mesh, PartitionSpec as P
    from jax.experimental.shard_map import shard_map

    devs = np.asarray(jax.devices()[:8]).reshape(B, HALVES)
    mesh = Mesh(devs, ("b", "s"))

    def shard_fn(x, maskf, W_qkv, a, W_qa, W_ak, W_out):
        # x: [1, 1, NS, DIM] local rows of one batch; maskf: [1, 1, NS] float 0/1
        x = x[0, 0]
        maskf = maskf[0, 0]
        qkv = (x @ W_qkv).reshape(NS, 3, HEADS, DIM_HEAD).transpose(1, 2, 0, 3)
        q, k, v = qkv[0], qkv[1], qkv[2]  # [h, NS, d]
        # qa path (fully local): [h, NS, m]
        qa_sim = jnp.einsum("hid,hjd->hij", q, a)
        qa_max = jnp.max(qa_sim, axis=-1, keepdims=True)
        qa_e = jnp.exp(qa_sim - qa_max)
        qa_attn = qa_e / jnp.sum(qa_e, axis=-1, keepdims=True)
        qa_attn = jnp.einsum("gh,hij->gij", W_qa, qa_attn)
        # ak path: [h, m, NS] local slice of n
        ak_sim = jnp.einsum("hid,hjd->hij", a, k)
        ak_e = jnp.exp(ak_sim) * maskf[None, None, :]
        z_part = jnp.sum(ak_e, axis=-1)  # [h, m]
        z = jax.lax.psum(z_part, "s")
        ak_f = ak_e / z[:, :, None]
        ak_f = jnp.einsum("gh,hij->gij", W_ak, ak_f)
        agent_part = jnp.einsum("hmn,hnd->hmd", ak_f, v)
        agent_out = jax.lax.psum(agent_part, "s")  # [h, m, d]
        out = jnp.einsum("hnm,hmd->hnd", qa_attn, agent_out)  # [h, NS, d]
        out = out * maskf[None, :, None]
        out = out.transpose(1, 0, 2).reshape(NS, DIM_INNER)
        return (out @ W_out)[None, None]  # [1, 1, NS, DIM]

    fn = shard_map(
        shard_fn,
        mesh=mesh,
        in_specs=(P("b", "s"), P("b", "s"), P(), P(), P(), P(), P()),
        out_specs=P("b", "s"),
        check_rep=False,
    )

    def run(x, mask, W_qkv, agent_tokens, W_qa, W_ak, W_out):
        xr = x.reshape(B, HALVES, NS, DIM)
        mr = mask.astype(np.float32).reshape(B, HALVES, NS)
        a = agent_tokens * SCALE
        out = jax.jit(fn)(xr, mr, W_qkv, a, W_qa, W_ak, W_out)
        return np.asarray(out).reshape(B, N, DIM)

    return run


_RUN = None


def _numpy_fallback(x, mask, W_qkv, agent_tokens, W_qa, W_ak, W_out):
    b, n, _ = x.shape
    out = np.empty((b, n, DIM), np.float32)
    a = (agent_tokens * SCALE).astype(np.float32)
    for bi in range(b):
        qkv = (x[bi] @ W_qkv).reshape(n, 3, HEADS, DIM_HEAD).transpose(1, 2, 0, 3)
        q, k, v = qkv[0], qkv[1], qkv[2]
        qa = np.einsum("hid,hjd->hij", q, a)
        qa = np.exp(qa - qa.max(-1, keepdims=True))
        qa /= qa.sum(-1, keepdims=True)
        qa = np.einsum("gh,hij->gij", W_qa, qa)
        ak = np.einsum("hid,hjd->hij", a, k)
        ak = np.exp(ak - ak.max(-1, keepdims=True)) * mask[bi].astype(np.float32)[None, None, :]
        ak /= ak.sum(-1, keepdims=True)
        ak = np.einsum("gh,hij->gij", W_ak, ak)
        agent = np.einsum("hmn,hnd->hmd", ak, v)
        o = np.einsum("hnm,hmd->hnd", qa, agent)
        o *= mask[bi].astype(np.float32)[None, :, None]
        out[bi] = o.transpose(1, 0, 2).reshape(n, DIM_INNER) @ W_out
    return out


def kernel(x, mask, W_qkv, agent_tokens, W_qa, W_ak, W_out):
    global _RUN
    if _RUN is None:
        try:
            _RUN = _build()
        except Exception:
            _RUN = _numpy_fallback
    out = _RUN(
        np.asarray(x, np.float32),
        np.asarray(mask),
        np.asarray(W_qkv, np.float32),
        np.asarray(agent_tokens, np.float32),
        np.asarray(W_qa, np.float32),
        np.asarray(W_ak, np.float32),
        np.asarray(W_out, np.float32),
    )
    return out.astype(np.float32)

